# revision 17
# baseline (speedup 1.0000x reference)
"""Trainium2 Bass kernel for 16-head MHA (B=4, L=2048, D=1024, fp32 in/out).

Sharding: batch x head-group over 8 cores. Core c handles batch c//2 and
heads (c%2)*8 .. (c%2)*8+7 (Megatron column-parallel QKV, row-parallel Wo).
Each core computes a partial output projection; the host sums the two
partials per batch.

All matmul operands are bf16 (fp32 accumulate in PSUM). Measured on this
hardware, fp32r moving operands stream at ~half rate (442ns for N=512 vs
213ns bf16), so bf16 halves Tensor-engine time; rel err stays ~1e-3 vs
the 2e-2 gate.

Per-core device program (SPMD, identical on all cores):
  upfront: VH = v @ Wv_g.T  -> [2048, 8*65] bf16 (65th col of each head =
           1.0 so the AV matmul also yields the softmax denominator);
           KHT = Wk_g @ k.T -> [512, 2048]; Q projection for column 0.
  main loop over 512-wide query columns j, head pairs c, key chunks lk:
      one [128,1024] PSUM score tile holds head A in cols 0:512 and head B
      in cols 512:1024 (two K=64 bf16 matmuls packed in PE row groups 0-1 /
      2-3, running concurrently); one FD=1024 exp converts it to bf16
      attention weights (optionally the tail XDVE columns are computed on
      the Vector engine with a Schraudolph bit-trick exp to offload the
      Activation engine); two K=128 matmuls accumulate O_un (+ denominator
      row) per head. AV trails scores/exp by PIPE steps. The Q projection
      for column j+1 and the output projection for column j-1 are drip-fed
      ONE matmul per step so the PE never bursts/starves the ACT stream.
      finalize: 1/denom via reciprocal_approx_fast, broadcast across
      partitions via a DRAM-bounce DMA, O = O_un * (1/denom) on GpSimd.
"""

import sys

if "/opt/trn_rl_repo" not in sys.path:
    sys.path.insert(0, "/opt/trn_rl_repo")

import numpy as np

B, LQ, LV, D, H = 4, 2048, 2048, 1024, 16
DH = D // H            # 64
N_CORES = 8
H_LOC = H // 2         # 8 heads per core
HD_LOC = H_LOC * DH    # 512 head-dims per core
NKC = D // 128         # 8 contraction chunks for projections
NS = LV // 128         # 16 key chunks
NMC = HD_LOC // 128    # 4 head-dim chunks (head pairs)
JQ = 512               # query block width in attention
NJ = LQ // JQ          # 4 query columns
VW = DH + 1            # 65: per-head V width incl. ones column

XDVE = 192             # e-tile columns computed on DVE (Schraudolph exp)
# bf16-as-int16 Schraudolph: e = exp(0.125*s) = 2^(0.125*log2e*s);
# bf16 bits of 2^y ~= 128*y + 127*128; mean-centering correction 7.42.
SCH_A = 0.125 * 128 * 1.4426950408889634
SCH_B = 16256.0 - 7.42

_PROG_CACHE = {}


def build_program(iters=1, phases="abc", xdve=XDVE):
    import concourse.bass as bass
    import concourse.tile as tile
    from concourse import bacc, mybir

    F32 = mybir.dt.float32
    BF16 = mybir.dt.bfloat16
    I16 = mybir.dt.int16
    EXP = mybir.ActivationFunctionType.Exp
    MULT = mybir.AluOpType.mult
    ADD = mybir.AluOpType.add

    nc = bacc.Bacc("TRN2", target_bir_lowering=False, debug=False,
                   num_devices=N_CORES)

    qT = nc.dram_tensor("qT", [D, LQ], BF16, kind="ExternalInput").ap()
    kT = nc.dram_tensor("kT", [D, LV], BF16, kind="ExternalInput").ap()
    vT = nc.dram_tensor("vT", [D, LV], BF16, kind="ExternalInput").ap()
    wqT = nc.dram_tensor("wqT", [D, HD_LOC], BF16, kind="ExternalInput").ap()
    wkT = nc.dram_tensor("wkT", [D, HD_LOC], BF16, kind="ExternalInput").ap()
    wvT = nc.dram_tensor("wvT", [D, HD_LOC], BF16, kind="ExternalInput").ap()
    woT = nc.dram_tensor("woT", [HD_LOC, D], BF16, kind="ExternalInput").ap()
    outT = nc.dram_tensor("outT", [D, LQ], F32, kind="ExternalOutput").ap()
    # DRAM bounce rows for broadcasting softmax 1/denom across partitions
    dbc = nc.dram_tensor("dbc", [2 * NMC * NJ, JQ], F32).ap()
    dbc2 = nc.dram_tensor("dbc2", [2 * NMC * NJ, JQ], F32).ap()

    with tile.TileContext(nc) as tc:
        with (
            tc.tile_pool(name="persist", bufs=1) as persist,
            tc.tile_pool(name="wq", bufs=1) as wqp,
            tc.tile_pool(name="wk", bufs=1) as wkp,
            tc.tile_pool(name="qact", bufs=3) as qactp,
            tc.tile_pool(name="kact", bufs=1) as kactp,
            tc.tile_pool(name="qhtj", bufs=8) as qhtp,
            tc.tile_pool(name="ohtj", bufs=8) as ohtp,
            tc.tile_pool(name="e", bufs=6) as epool,
            tc.tile_pool(name="smalls", bufs=4) as smalls,
            tc.tile_pool(name="rbcp", bufs=2) as rbcp,
            tc.tile_pool(name="wo", bufs=1) as wop,
            tc.tile_pool(name="outp", bufs=6) as outp,
            tc.tile_pool(name="pss", bufs=2, space="PSUM") as pss,
            tc.tile_pool(name="psav", bufs=2, space="PSUM") as psav,
            tc.tile_pool(name="pspj", bufs=2, space="PSUM") as pspj,
        ):
            def body():
                kht = [persist.tile([128, LV], BF16, tag=f"kht{m}", name=f"kht{m}")
                       for m in range(NMC)]
                vh = [persist.tile([128, H_LOC * VW], BF16, tag=f"vh{s}", name=f"vh{s}")
                      for s in range(NS)]
                ones8 = persist.tile([128, H_LOC], BF16, tag="ones8", name="ones8")
                nc.vector.memset(ones8[:], 1.0)

                W = {}
                qht = {}   # (m, n) -> [128, JQ] tile
                oht = {}   # (c, j) -> [128, JQ] tile

                def load_qact(n):
                    t = qactp.tile([128, NKC, 512], BF16, tag="qact",
                                   name=f"qx{n}")
                    for h in range(2):
                        a = h * (NKC // 2)
                        nc.sync.dma_start(
                            t[:, a:a + NKC // 2, :],
                            bass.AP(tensor=qT.tensor,
                                    offset=a * 128 * LQ + n * 512,
                                    ap=[[LQ, 128], [128 * LQ, NKC // 2],
                                        [1, 512]]))
                    return t

                def qproj_chain_mms(xs, n, m):
                    # one closure per matmul so chains can be drip-fed one
                    # MM per attention step
                    state = {}
                    mms = []
                    for kc in range(NKC):
                        def mm(kc=kc):
                            if kc == 0:
                                state["p"] = pspj.tile([128, 512], F32,
                                                       tag="pspj",
                                                       name=f"qp{n}_{m}")
                            nc.tensor.matmul(
                                state["p"][:],
                                W["q"][:, kc, m * 128:(m + 1) * 128],
                                xs[:, kc, :],
                                start=(kc == 0), stop=(kc == NKC - 1))
                            if kc == NKC - 1:
                                d = qhtp.tile([128, JQ], BF16, tag="qhtj",
                                              name=f"qh{n}_{m}")
                                nc.vector.tensor_copy(out=d[:], in_=state["p"][:])
                                qht[(m, n)] = d
                        mms.append(mm)
                    return mms

                def kproj_chain_mms(kx, n, m):
                    state = {}
                    mms = []
                    for kc in range(NKC):
                        def mm(kc=kc):
                            if kc == 0:
                                state["p"] = pspj.tile([128, 512], F32,
                                                       tag="pspj",
                                                       name=f"kp{n}_{m}")
                            nc.tensor.matmul(
                                state["p"][:],
                                W["k"][:, kc, m * 128:(m + 1) * 128],
                                kx[:, kc, n * 512:(n + 1) * 512],
                                start=(kc == 0), stop=(kc == NKC - 1))
                            if kc == NKC - 1:
                                nc.vector.tensor_copy(
                                    out=kht[m][:, n * 512:(n + 1) * 512],
                                    in_=state["p"][:])
                        mms.append(mm)
                    return mms

                def outproj_chain_mms(j, m):
                    state = {}
                    NWO = HD_LOC // 128
                    mms = []
                    for kc in range(NWO):
                        def mm(kc=kc):
                            if kc == 0:
                                state["p"] = pspj.tile([128, 512], F32,
                                                       tag="pspj",
                                                       name=f"cp{j}_{m}")
                            nc.tensor.matmul(
                                state["p"][:],
                                W["o"][:, kc, m * 128:(m + 1) * 128],
                                oht[(kc, j)][:],
                                start=(kc == 0), stop=(kc == NWO - 1))
                            if kc == NWO - 1:
                                om = outp.tile([128, JQ], F32, tag="om",
                                               name=f"om{j}_{m}")
                                nc.vector.tensor_copy(out=om[:], in_=state["p"][:])
                                eng = (nc.sync if (m % 2 == 0 or j == NJ - 1)
                                       else nc.gpsimd)
                                for q2 in range(2):
                                    eng.dma_start(
                                        outT[m * 128:(m + 1) * 128,
                                             j * JQ + q2 * 256:
                                             j * JQ + (q2 + 1) * 256],
                                        om[:, q2 * 256:(q2 + 1) * 256])
                                if m == D // 128 - 1:
                                    for c in range(NMC):
                                        oht.pop((c, j))
                        mms.append(mm)
                    return mms

                qxs = {0: None}

                def wload(pool, name, drt, nkc, width, pieces=2):
                    t = pool.tile([128, nkc, width], BF16, tag=name,
                                  name=name)
                    step = nkc // pieces
                    for h in range(pieces):
                        a = h * step
                        nc.sync.dma_start(
                            t[:, a:a + step, :],
                            bass.AP(tensor=drt.tensor,
                                    offset=a * 128 * width,
                                    ap=[[width, 128],
                                        [128 * width, step],
                                        [1, width]]))
                    return t

                # ---------- upfront: V projection, K projection, Q col 0 ----
                with (tc.tile_pool(name="wproj", bufs=1) as wpool,
                      tc.tile_pool(name="vact", bufs=4) as vactp):
                    psA = pspj
                    wvx = wload(wpool, "wv", wvT, NKC, HD_LOC, pieces=NKC)

                    def load_vx(sg):
                        vx = vactp.tile([128, NKC, 512], BF16, tag="vact",
                                        name=f"vx{sg}")
                        for a in range(NKC):
                            nc.sync.dma_start(
                                vx[:, a, :],
                                vT[a * 128:(a + 1) * 128,
                                   sg * 512:(sg + 1) * 512])
                        return vx

                    vxs = [load_vx(sg) for sg in range(NS // 4)]
                    W["k"] = wload(wkp, "wk", wkT, NKC, HD_LOC)
                    # k: one persistent [128, NKC, LV] tile, 1 DMA per kc
                    kx = kactp.tile([128, NKC, LV], BF16, tag="kx", name="kx")
                    for kc in range(NKC):
                        nc.sync.dma_start(kx[:, kc, :],
                                          kT[kc * 128:(kc + 1) * 128, :])
                    for sg in range(NS // 4):
                        vx = vxs[sg]
                        for si in range(4):
                            s = sg * 4 + si
                            p = psA.tile([128, HD_LOC], F32, tag="pspj", name=f"pv{s}")
                            for kc in range(NKC):
                                nc.tensor.matmul(
                                    p[:], vx[:, kc, si * 128:(si + 1) * 128],
                                    wvx[:, kc, :],
                                    start=(kc == 0), stop=(kc == NKC - 1))
                            v3 = vh[s].rearrange("p (h e) -> p h e", e=VW)
                            nc.vector.tensor_copy(
                                out=v3[:, :, DH:VW],
                                in_=ones8.rearrange("p (h o) -> p h o", o=1))
                            nc.vector.tensor_copy(
                                out=v3[:, :, 0:DH],
                                in_=p.rearrange("p (h e) -> p h e", e=DH))

                    # K projection m=0,1 upfront (m=2,3 drip-fed in column 0)
                    for n in range(LQ // 512):
                        for m in range(2):
                            p = psA.tile([128, 512], F32, tag="pspj",
                                         name=f"kp{n}_{m}")
                            for kc in range(NKC):
                                nc.tensor.matmul(
                                    p[:],
                                    W["k"][:, kc, m * 128:(m + 1) * 128],
                                    kx[:, kc, n * 512:(n + 1) * 512],
                                    start=(kc == 0), stop=(kc == NKC - 1))
                            nc.vector.tensor_copy(
                                out=kht[m][:, n * 512:(n + 1) * 512], in_=p[:])

                    # Q projection for column 0
                    W["q"] = wload(wqp, "wq", wqT, NKC, HD_LOC)
                    xs0 = load_qact(0)
                    W["o"] = wload(wop, "wo", woT, HD_LOC // 128, D)
                    for m in range(NMC):
                        p = psA.tile([128, 512], F32, tag="pspj", name=f"qp0_{m}")
                        for kc in range(NKC):
                            nc.tensor.matmul(
                                p[:],
                                W["q"][:, kc, m * 128:(m + 1) * 128],
                                xs0[:, kc, :],
                                start=(kc == 0), stop=(kc == NKC - 1))
                        d = qhtp.tile([128, JQ], BF16, tag="qhtj", name=f"qh0_{m}")
                        nc.vector.tensor_copy(out=d[:], in_=p[:])
                        qht[(m, 0)] = d
                    if NJ > 1:
                        qxs[1] = load_qact(1)

                if "b" not in phases:
                    nc.sync.dma_start(outT[0:128, 0:1024],
                                      kht[0][:, 0:2048].bitcast(F32))
                    return

                # ---------- main loop ----------
                PIPE = 4
                blocks = [(c, j) for j in range(NJ) for c in range(NMC)]
                steps = [(bi, lk) for bi in range(len(blocks))
                         for lk in range(NS)]
                psx = {}
                ets = {}
                budget = [0.0]

                def emit_scores(bi, lk):
                    c, j = blocks[bi]
                    st = pss.tile([128, 2 * JQ], F32, tag="pss",
                                  name=f"st{bi}_{lk}")
                    nc.tensor.matmul(
                        st[:, 0:JQ],
                        kht[c][0:64, lk * 128:(lk + 1) * 128],
                        qht[(c, j)][0:64, :])
                    nc.tensor.matmul(
                        st[:, JQ:2 * JQ],
                        kht[c][64:128, lk * 128:(lk + 1) * 128],
                        qht[(c, j)][64:128, :])
                    e = epool.tile([128, 2 * JQ], BF16, tag="e",
                                   name=f"e{bi}_{lk}")
                    if xdve:
                        # alternate the approx window between the two tile
                        # edges so every query mixes exact and approx weights
                        if lk % 2 == 0:
                            dlo, dhi = 0, xdve
                            alo, ahi = xdve, 2 * JQ
                        else:
                            dlo, dhi = 2 * JQ - xdve, 2 * JQ
                            alo, ahi = 0, 2 * JQ - xdve
                        nc.scalar.activation(e[:, alo:ahi], st[:, alo:ahi],
                                             EXP, scale=0.125)
                        nc.vector.tensor_scalar(
                            out=e[:, dlo:dhi].bitcast(I16),
                            in0=st[:, dlo:dhi],
                            scalar1=SCH_A, scalar2=SCH_B,
                            op0=MULT, op1=ADD)
                    else:
                        nc.scalar.activation(e[:], st[:], EXP, scale=0.125)
                    ets[(bi, lk)] = e

                def emit_av(bi, lk):
                    c, j = blocks[bi]
                    hA, hB = 2 * c, 2 * c + 1
                    if lk == 0:
                        psx[bi] = (
                            psav.tile([VW, JQ], F32, tag="psav", name=f"pa{bi}"),
                            psav.tile([VW, JQ], F32, tag="psav", name=f"pb{bi}"),
                        )
                    e = ets.pop((bi, lk))
                    pA, pB = psx[bi]
                    nc.tensor.matmul(
                        pA[:], vh[lk][:, hA * VW:(hA + 1) * VW], e[:, 0:JQ],
                        start=(lk == 0), stop=(lk == NS - 1))
                    nc.tensor.matmul(
                        pB[:], vh[lk][:, hB * VW:(hB + 1) * VW], e[:, JQ:2 * JQ],
                        start=(lk == 0), stop=(lk == NS - 1))
                    if lk == NS - 1:
                        emit_finalize(bi)

                def emit_finalize(bi):
                    c, j = blocks[bi]
                    oh = ohtp.tile([128, JQ], BF16, tag="ohtj", name=f"oh{bi}")
                    oht[(c, j)] = oh
                    ouns = []
                    for hi, px in enumerate(psx.pop(bi)):
                        oun = rbcp.tile([VW, JQ], F32, tag="oun",
                                        name=f"ou{bi}_{hi}")
                        nc.vector.tensor_copy(out=oun[:], in_=px[:])
                        di = bi * 2 + hi
                        nc.sync.dma_start(dbc[di:di + 1, :], oun[DH:VW, :])
                        ouns.append(oun)
                    # 1/denom with queries on partitions: flat [128, 8] view
                    # of both rows so the reciprocal has free-dim 8, not 512
                    dt_ = smalls.tile([128, 8], F32, tag="dt", name=f"dt{bi}")
                    nc.sync.dma_start(
                        dt_[:], bass.AP(tensor=dbc.tensor, offset=2 * bi * JQ,
                                        ap=[[8, 128], [1, 8]]))
                    rt = smalls.tile([128, 8], F32, tag="rt", name=f"rt{bi}")
                    nc.vector.reciprocal(rt[:], dt_[:])
                    nc.sync.dma_start(
                        bass.AP(tensor=dbc2.tensor, offset=2 * bi * JQ,
                                ap=[[8, 128], [1, 8]]), rt[:])
                    mul_eng = (nc.vector if bi == NMC * NJ - 1
                               else nc.gpsimd)
                    for hi in range(2):
                        di = bi * 2 + hi
                        rbc = rbcp.tile([64, JQ], F32, tag="rbc",
                                        name=f"rb{bi}_{hi}")
                        bc_src = bass.AP(tensor=dbc2.tensor, offset=di * JQ,
                                         ap=[[0, 64], [1, JQ]])
                        nc.sync.dma_start(rbc[:], bc_src)
                        mul_eng.tensor_mul(
                            oh[hi * 64:(hi + 1) * 64, :],
                            ouns[hi][0:DH, :], rbc[:])

                do_c = "c" in phases
                work = []   # drip-fed projection matmuls, one per step
                for t in range(len(steps) + PIPE):
                    if t < len(steps):
                        emit_scores(*steps[t])
                    if t >= PIPE:
                        bi, lk = steps[t - PIPE]
                        c, j = blocks[bi]
                        emit_av(bi, lk)
                        sl = (bi % NMC) * NS + lk   # step within column
                        if sl == NS and j + 2 < NJ:
                            # prefetch column j+2's activations early
                            qxs[j + 2] = load_qact(j + 2)
                        if sl == 0:
                            if j == 0:
                                for m in (2, 3):
                                    for n in range(LQ // 512):
                                        work.extend(
                                            kproj_chain_mms(kx, n, m))
                            if j < NJ - 1 and (j + 1) not in qxs:
                                qxs[j + 1] = load_qact(j + 1)
                            if j < NJ - 1:
                                for m in range(NMC):
                                    work.extend(
                                        qproj_chain_mms(qxs[j + 1], j + 1, m))
                            if do_c and j >= 1:
                                for m in range(D // 128):
                                    work.extend(outproj_chain_mms(j - 1, m))
                            budget[0] = 0.0
                        budget[0] += 1.5 if j == 0 else 1.0
                        while budget[0] >= 1.0 and work:
                            work.pop(0)()
                            budget[0] -= 1.0
                if do_c:
                    while work:
                        work.pop(0)()
                    for m in range(D // 128):
                        for mm in outproj_chain_mms(NJ - 1, m):
                            mm()
                else:
                    for (c, j), oh in sorted(oht.items()):
                        nc.sync.dma_start(
                            outT[c * 128:(c + 1) * 128,
                                 j * JQ:(j + 1) * JQ],
                            oh[:, 0:JQ // 2].bitcast(F32))

            if iters == 1:
                body()
            else:
                with tc.For_i(0, iters, 1):
                    body()

    nc.compile()
    return nc


def get_program(iters=1, phases="abc", xdve=XDVE):
    key = (iters, phases, xdve)
    if key not in _PROG_CACHE:
        _PROG_CACHE[key] = build_program(iters, phases, xdve)
    return _PROG_CACHE[key]


def shard_inputs(q, k, v, Wq, Wk, Wv, Wo):
    """Build the 8 per-core input maps (host-side layout prep only)."""
    import ml_dtypes
    BF = ml_dtypes.bfloat16
    q, k, v = (np.asarray(x, np.float32) for x in (q, k, v))
    Wq, Wk, Wv, Wo = (np.asarray(x, np.float32) for x in (Wq, Wk, Wv, Wo))
    in_maps = []
    for core in range(N_CORES):
        b, g = core // 2, core % 2
        rows = slice(g * HD_LOC, (g + 1) * HD_LOC)
        in_maps.append({
            "qT": np.ascontiguousarray(q[b].T.astype(BF)),
            "kT": np.ascontiguousarray(k[b].T.astype(BF)),
            "vT": np.ascontiguousarray(v[b].T.astype(BF)),
            "wqT": np.ascontiguousarray(Wq[rows, :].T.astype(BF)),
            "wkT": np.ascontiguousarray(Wk[rows, :].T.astype(BF)),
            "wvT": np.ascontiguousarray(Wv[rows, :].T.astype(BF)),
            "woT": np.ascontiguousarray(Wo[:, rows].T.astype(BF)),
        })
    return in_maps


def gather_outputs(results):
    out = np.empty((B, LQ, D), np.float32)
    for b in range(B):
        acc = results[2 * b]["outT"] + results[2 * b + 1]["outT"]
        out[b] = acc.T
    return out


def kernel(q, k, v, Wq, Wk, Wv, Wo):
    from concourse.bass_utils import run_bass_kernel_spmd

    nc = get_program(1)
    in_maps = shard_inputs(q, k, v, Wq, Wk, Wv, Wo)
    res = run_bass_kernel_spmd(nc, in_maps, core_ids=list(range(N_CORES)))
    return gather_outputs(res.results)


# revision 19
# speedup vs baseline: 1.0121x; 1.0121x over previous
"""Trainium2 Bass kernel for 16-head MHA (B=4, L=2048, D=1024, fp32 in/out).

Sharding: batch x head-group over 8 cores. Core c handles batch c//2 and
heads (c%2)*8 .. (c%2)*8+7 (Megatron column-parallel QKV, row-parallel Wo).
Each core computes a partial output projection; the host sums the two
partials per batch.

All matmul operands are bf16 (fp32 accumulate in PSUM). Measured on this
hardware, fp32r moving operands stream at ~half rate (442ns for N=512 vs
213ns bf16), so bf16 halves Tensor-engine time; rel err stays ~1e-3 vs
the 2e-2 gate.

Per-core device program (SPMD, identical on all cores):
  upfront: VH = v @ Wv_g.T  -> [2048, 8*65] bf16 (65th col of each head =
           1.0 so the AV matmul also yields the softmax denominator);
           KHT = Wk_g @ k.T -> [512, 2048]; Q projection for column 0.
  main loop over 512-wide query columns j, head pairs c, key chunks lk:
      one [128,1024] PSUM score tile holds head A in cols 0:512 and head B
      in cols 512:1024 (two K=64 bf16 matmuls packed in PE row groups 0-1 /
      2-3, running concurrently); one FD=1024 exp converts it to bf16
      attention weights (optionally the tail XDVE columns are computed on
      the Vector engine with a Schraudolph bit-trick exp to offload the
      Activation engine); two K=128 matmuls accumulate O_un (+ denominator
      row) per head. AV trails scores/exp by PIPE steps. The Q projection
      for column j+1 and the output projection for column j-1 are drip-fed
      ONE matmul per step so the PE never bursts/starves the ACT stream
      (column 0 additionally absorbs the K projection for head pairs 2-3
      at 1.5 matmuls per step).
      finalize: softmax denominators bounce through DRAM and come back
      with queries on partitions ([128, 8]) so the Vector-engine
      reciprocal runs at free-dim 8 (~0.2us, not 3.3us); the broadcast
      back across 64 partitions is a second DRAM-bounce DMA;
      O = O_un * (1/denom) runs on GpSimd (SBUF-only engine, otherwise
      idle). Input DMAs are batched into ~128-512KB pieces: big enough
      to amortize the ~585ns per-DMA Sync-engine issue cost, small
      enough to spread transfers across the 16 DMA engines.
"""

import sys

if "/opt/trn_rl_repo" not in sys.path:
    sys.path.insert(0, "/opt/trn_rl_repo")

import numpy as np

B, LQ, LV, D, H = 4, 2048, 2048, 1024, 16
DH = D // H            # 64
N_CORES = 8
H_LOC = H // 2         # 8 heads per core
HD_LOC = H_LOC * DH    # 512 head-dims per core
NKC = D // 128         # 8 contraction chunks for projections
NS = LV // 128         # 16 key chunks
NMC = HD_LOC // 128    # 4 head-dim chunks (head pairs)
JQ = 512               # query block width in attention
NJ = LQ // JQ          # 4 query columns
VW = DH + 1            # 65: per-head V width incl. ones column

XDVE = 192             # e-tile columns computed on DVE (Schraudolph exp)
# bf16-as-int16 Schraudolph: e = exp(0.125*s) = 2^(0.125*log2e*s);
# bf16 bits of 2^y ~= 128*y + 127*128; mean-centering correction 7.42.
SCH_A = 0.125 * 128 * 1.4426950408889634
SCH_B = 16256.0 - 7.42

_PROG_CACHE = {}


def build_program(iters=1, phases="abc", xdve=XDVE):
    import concourse.bass as bass
    import concourse.tile as tile
    from concourse import bacc, mybir

    F32 = mybir.dt.float32
    BF16 = mybir.dt.bfloat16
    I16 = mybir.dt.int16
    EXP = mybir.ActivationFunctionType.Exp
    MULT = mybir.AluOpType.mult
    ADD = mybir.AluOpType.add

    nc = bacc.Bacc("TRN2", target_bir_lowering=False, debug=False,
                   num_devices=N_CORES)

    qT = nc.dram_tensor("qT", [D, LQ], BF16, kind="ExternalInput").ap()
    kT = nc.dram_tensor("kT", [D, LV], BF16, kind="ExternalInput").ap()
    vT = nc.dram_tensor("vT", [D, LV], BF16, kind="ExternalInput").ap()
    wqT = nc.dram_tensor("wqT", [D, HD_LOC], BF16, kind="ExternalInput").ap()
    wkT = nc.dram_tensor("wkT", [D, HD_LOC], BF16, kind="ExternalInput").ap()
    wvT = nc.dram_tensor("wvT", [D, HD_LOC], BF16, kind="ExternalInput").ap()
    woT = nc.dram_tensor("woT", [HD_LOC, D], BF16, kind="ExternalInput").ap()
    outT = nc.dram_tensor("outT", [D, LQ], F32, kind="ExternalOutput").ap()
    # DRAM bounce rows for broadcasting softmax 1/denom across partitions
    dbc = nc.dram_tensor("dbc", [2 * NMC * NJ, JQ], F32).ap()
    dbc2 = nc.dram_tensor("dbc2", [2 * NMC * NJ, JQ], F32).ap()

    with tile.TileContext(nc) as tc:
        with (
            tc.tile_pool(name="persist", bufs=1) as persist,
            tc.tile_pool(name="wq", bufs=1) as wqp,
            tc.tile_pool(name="wk", bufs=1) as wkp,
            tc.tile_pool(name="qact", bufs=3) as qactp,
            tc.tile_pool(name="kact", bufs=1) as kactp,
            tc.tile_pool(name="qhtj", bufs=8) as qhtp,
            tc.tile_pool(name="ohtj", bufs=8) as ohtp,
            tc.tile_pool(name="e", bufs=4) as epool,
            tc.tile_pool(name="smalls", bufs=4) as smalls,
            tc.tile_pool(name="rbcp", bufs=2) as rbcp,
            tc.tile_pool(name="wo", bufs=1) as wop,
            tc.tile_pool(name="outp", bufs=6) as outp,
            tc.tile_pool(name="pss", bufs=2, space="PSUM") as pss,
            tc.tile_pool(name="psav", bufs=2, space="PSUM") as psav,
            tc.tile_pool(name="pspj", bufs=2, space="PSUM") as pspj,
        ):
            def body():
                kht = [persist.tile([128, LV], BF16, tag=f"kht{m}", name=f"kht{m}")
                       for m in range(NMC)]
                vh = [persist.tile([128, H_LOC * VW], BF16, tag=f"vh{s}", name=f"vh{s}")
                      for s in range(NS)]
                ones8 = persist.tile([128, H_LOC], BF16, tag="ones8", name="ones8")
                nc.vector.memset(ones8[:], 1.0)

                W = {}
                qht = {}   # (m, n) -> [128, JQ] tile
                oht = {}   # (c, j) -> [128, JQ] tile

                def load_qact(n):
                    t = qactp.tile([128, NKC, 512], BF16, tag="qact",
                                   name=f"qx{n}")
                    for h in range(2):
                        a = h * (NKC // 2)
                        nc.sync.dma_start(
                            t[:, a:a + NKC // 2, :],
                            bass.AP(tensor=qT.tensor,
                                    offset=a * 128 * LQ + n * 512,
                                    ap=[[LQ, 128], [128 * LQ, NKC // 2],
                                        [1, 512]]))
                    return t

                def qproj_chain_mms(xs, n, m):
                    # one closure per matmul so chains can be drip-fed one
                    # MM per attention step
                    state = {}
                    mms = []
                    for kc in range(NKC):
                        def mm(kc=kc):
                            if kc == 0:
                                state["p"] = pspj.tile([128, 512], F32,
                                                       tag="pspj",
                                                       name=f"qp{n}_{m}")
                            nc.tensor.matmul(
                                state["p"][:],
                                W["q"][:, kc, m * 128:(m + 1) * 128],
                                xs[:, kc, :],
                                start=(kc == 0), stop=(kc == NKC - 1))
                            if kc == NKC - 1:
                                d = qhtp.tile([128, JQ], BF16, tag="qhtj",
                                              name=f"qh{n}_{m}")
                                nc.vector.tensor_copy(out=d[:], in_=state["p"][:])
                                qht[(m, n)] = d
                        mms.append(mm)
                    return mms

                def kproj_chain_mms(kx, n, m):
                    state = {}
                    mms = []
                    for kc in range(NKC):
                        def mm(kc=kc):
                            if kc == 0:
                                state["p"] = pspj.tile([128, 512], F32,
                                                       tag="pspj",
                                                       name=f"kp{n}_{m}")
                            nc.tensor.matmul(
                                state["p"][:],
                                W["k"][:, kc, m * 128:(m + 1) * 128],
                                kx[:, kc, n * 512:(n + 1) * 512],
                                start=(kc == 0), stop=(kc == NKC - 1))
                            if kc == NKC - 1:
                                nc.vector.tensor_copy(
                                    out=kht[m][:, n * 512:(n + 1) * 512],
                                    in_=state["p"][:])
                        mms.append(mm)
                    return mms

                def outproj_chain_mms(j, m):
                    state = {}
                    NWO = HD_LOC // 128
                    mms = []
                    for kc in range(NWO):
                        def mm(kc=kc):
                            if kc == 0:
                                state["p"] = pspj.tile([128, 512], F32,
                                                       tag="pspj",
                                                       name=f"cp{j}_{m}")
                            nc.tensor.matmul(
                                state["p"][:],
                                W["o"][:, kc, m * 128:(m + 1) * 128],
                                oht[(kc, j)][:],
                                start=(kc == 0), stop=(kc == NWO - 1))
                            if kc == NWO - 1:
                                om = outp.tile([128, JQ], F32, tag="om",
                                               name=f"om{j}_{m}")
                                nc.vector.tensor_copy(out=om[:], in_=state["p"][:])
                                eng = (nc.sync if (m % 2 == 0 or j == NJ - 1)
                                       else nc.gpsimd)
                                for q2 in range(2):
                                    eng.dma_start(
                                        outT[m * 128:(m + 1) * 128,
                                             j * JQ + q2 * 256:
                                             j * JQ + (q2 + 1) * 256],
                                        om[:, q2 * 256:(q2 + 1) * 256])
                                if m == D // 128 - 1:
                                    for c in range(NMC):
                                        oht.pop((c, j))
                        mms.append(mm)
                    return mms

                qxs = {0: None}

                def wload(pool, name, drt, nkc, width, pieces=2):
                    t = pool.tile([128, nkc, width], BF16, tag=name,
                                  name=name)
                    step = nkc // pieces
                    for h in range(pieces):
                        a = h * step
                        nc.sync.dma_start(
                            t[:, a:a + step, :],
                            bass.AP(tensor=drt.tensor,
                                    offset=a * 128 * width,
                                    ap=[[width, 128],
                                        [128 * width, step],
                                        [1, width]]))
                    return t

                # ---------- upfront: V projection, K projection, Q col 0 ----
                with (tc.tile_pool(name="wproj", bufs=1) as wpool,
                      tc.tile_pool(name="vact", bufs=4) as vactp):
                    psA = pspj
                    wvx = wload(wpool, "wv", wvT, NKC, HD_LOC, pieces=NKC)

                    def load_vx(sg):
                        vx = vactp.tile([128, NKC, 512], BF16, tag="vact",
                                        name=f"vx{sg}")
                        for a in range(NKC):
                            nc.sync.dma_start(
                                vx[:, a, :],
                                vT[a * 128:(a + 1) * 128,
                                   sg * 512:(sg + 1) * 512])
                        return vx

                    vxs = [load_vx(sg) for sg in range(NS // 4)]
                    W["k"] = wload(wkp, "wk", wkT, NKC, HD_LOC)
                    # k: one persistent [128, NKC, LV] tile, 1 DMA per kc
                    kx = kactp.tile([128, NKC, LV], BF16, tag="kx", name="kx")
                    for kc in range(NKC):
                        nc.sync.dma_start(kx[:, kc, :],
                                          kT[kc * 128:(kc + 1) * 128, :])
                    for sg in range(NS // 4):
                        vx = vxs[sg]
                        for si in range(4):
                            s = sg * 4 + si
                            p = psA.tile([128, HD_LOC], F32, tag="pspj", name=f"pv{s}")
                            for kc in range(NKC):
                                nc.tensor.matmul(
                                    p[:], vx[:, kc, si * 128:(si + 1) * 128],
                                    wvx[:, kc, :],
                                    start=(kc == 0), stop=(kc == NKC - 1))
                            v3 = vh[s].rearrange("p (h e) -> p h e", e=VW)
                            nc.vector.tensor_copy(
                                out=v3[:, :, DH:VW],
                                in_=ones8.rearrange("p (h o) -> p h o", o=1))
                            nc.vector.tensor_copy(
                                out=v3[:, :, 0:DH],
                                in_=p.rearrange("p (h e) -> p h e", e=DH))

                    # K projection m=0,1 upfront (m=2,3 drip-fed in column 0)
                    for n in range(LQ // 512):
                        for m in range(2):
                            p = psA.tile([128, 512], F32, tag="pspj",
                                         name=f"kp{n}_{m}")
                            for kc in range(NKC):
                                nc.tensor.matmul(
                                    p[:],
                                    W["k"][:, kc, m * 128:(m + 1) * 128],
                                    kx[:, kc, n * 512:(n + 1) * 512],
                                    start=(kc == 0), stop=(kc == NKC - 1))
                            nc.vector.tensor_copy(
                                out=kht[m][:, n * 512:(n + 1) * 512], in_=p[:])

                    # Q projection for column 0
                    W["q"] = wload(wqp, "wq", wqT, NKC, HD_LOC)
                    xs0 = load_qact(0)
                    W["o"] = wload(wop, "wo", woT, HD_LOC // 128, D)
                    for m in range(NMC):
                        p = psA.tile([128, 512], F32, tag="pspj", name=f"qp0_{m}")
                        for kc in range(NKC):
                            nc.tensor.matmul(
                                p[:],
                                W["q"][:, kc, m * 128:(m + 1) * 128],
                                xs0[:, kc, :],
                                start=(kc == 0), stop=(kc == NKC - 1))
                        d = qhtp.tile([128, JQ], BF16, tag="qhtj", name=f"qh0_{m}")
                        nc.vector.tensor_copy(out=d[:], in_=p[:])
                        qht[(m, 0)] = d
                    if NJ > 1:
                        qxs[1] = load_qact(1)

                if "b" not in phases:
                    nc.sync.dma_start(outT[0:128, 0:1024],
                                      kht[0][:, 0:2048].bitcast(F32))
                    return

                # ---------- main loop ----------
                PIPE = 3
                blocks = [(c, j) for j in range(NJ) for c in range(NMC)]
                steps = [(bi, lk) for bi in range(len(blocks))
                         for lk in range(NS)]
                psx = {}
                ets = {}
                budget = [0.0]

                def emit_scores(bi, lk):
                    c, j = blocks[bi]
                    st = pss.tile([128, 2 * JQ], F32, tag="pss",
                                  name=f"st{bi}_{lk}")
                    nc.tensor.matmul(
                        st[:, 0:JQ],
                        kht[c][0:64, lk * 128:(lk + 1) * 128],
                        qht[(c, j)][0:64, :])
                    nc.tensor.matmul(
                        st[:, JQ:2 * JQ],
                        kht[c][64:128, lk * 128:(lk + 1) * 128],
                        qht[(c, j)][64:128, :])
                    e = epool.tile([128, 2 * JQ], BF16, tag="e",
                                   name=f"e{bi}_{lk}")
                    if xdve:
                        # alternate the approx window between the two tile
                        # edges so every query mixes exact and approx weights
                        if lk % 2 == 0:
                            dlo, dhi = 0, xdve
                            alo, ahi = xdve, 2 * JQ
                        else:
                            dlo, dhi = 2 * JQ - xdve, 2 * JQ
                            alo, ahi = 0, 2 * JQ - xdve
                        nc.scalar.activation(e[:, alo:ahi], st[:, alo:ahi],
                                             EXP, scale=0.125)
                        nc.vector.tensor_scalar(
                            out=e[:, dlo:dhi].bitcast(I16),
                            in0=st[:, dlo:dhi],
                            scalar1=SCH_A, scalar2=SCH_B,
                            op0=MULT, op1=ADD)
                    else:
                        nc.scalar.activation(e[:], st[:], EXP, scale=0.125)
                    ets[(bi, lk)] = e

                def emit_av(bi, lk):
                    c, j = blocks[bi]
                    hA, hB = 2 * c, 2 * c + 1
                    if lk == 0:
                        psx[bi] = (
                            psav.tile([VW, JQ], F32, tag="psav", name=f"pa{bi}"),
                            psav.tile([VW, JQ], F32, tag="psav", name=f"pb{bi}"),
                        )
                    e = ets.pop((bi, lk))
                    pA, pB = psx[bi]
                    nc.tensor.matmul(
                        pA[:], vh[lk][:, hA * VW:(hA + 1) * VW], e[:, 0:JQ],
                        start=(lk == 0), stop=(lk == NS - 1))
                    nc.tensor.matmul(
                        pB[:], vh[lk][:, hB * VW:(hB + 1) * VW], e[:, JQ:2 * JQ],
                        start=(lk == 0), stop=(lk == NS - 1))
                    if lk == NS - 1:
                        emit_finalize(bi)

                def emit_finalize(bi):
                    c, j = blocks[bi]
                    oh = ohtp.tile([128, JQ], BF16, tag="ohtj", name=f"oh{bi}")
                    oht[(c, j)] = oh
                    ouns = []
                    for hi, px in enumerate(psx.pop(bi)):
                        oun = rbcp.tile([VW, JQ], F32, tag="oun",
                                        name=f"ou{bi}_{hi}")
                        nc.vector.tensor_copy(out=oun[:], in_=px[:])
                        di = bi * 2 + hi
                        nc.sync.dma_start(dbc[di:di + 1, :], oun[DH:VW, :])
                        ouns.append(oun)
                    # 1/denom with queries on partitions: flat [128, 8] view
                    # of both rows so the reciprocal has free-dim 8, not 512
                    dt_ = smalls.tile([128, 8], F32, tag="dt", name=f"dt{bi}")
                    nc.sync.dma_start(
                        dt_[:], bass.AP(tensor=dbc.tensor, offset=2 * bi * JQ,
                                        ap=[[8, 128], [1, 8]]))
                    rt = smalls.tile([128, 8], F32, tag="rt", name=f"rt{bi}")
                    nc.vector.reciprocal(rt[:], dt_[:])
                    nc.sync.dma_start(
                        bass.AP(tensor=dbc2.tensor, offset=2 * bi * JQ,
                                ap=[[8, 128], [1, 8]]), rt[:])
                    mul_eng = (nc.vector if bi == NMC * NJ - 1
                               else nc.gpsimd)
                    for hi in range(2):
                        di = bi * 2 + hi
                        rbc = rbcp.tile([64, JQ], F32, tag="rbc",
                                        name=f"rb{bi}_{hi}")
                        bc_src = bass.AP(tensor=dbc2.tensor, offset=di * JQ,
                                         ap=[[0, 64], [1, JQ]])
                        nc.sync.dma_start(rbc[:], bc_src)
                        mul_eng.tensor_mul(
                            oh[hi * 64:(hi + 1) * 64, :],
                            ouns[hi][0:DH, :], rbc[:])

                do_c = "c" in phases
                work = []   # drip-fed projection matmuls, one per step
                for t in range(len(steps) + PIPE):
                    if t < len(steps):
                        emit_scores(*steps[t])
                    if t >= PIPE:
                        bi, lk = steps[t - PIPE]
                        c, j = blocks[bi]
                        emit_av(bi, lk)
                        sl = (bi % NMC) * NS + lk   # step within column
                        if sl == NS and j + 2 < NJ:
                            # prefetch column j+2's activations early
                            qxs[j + 2] = load_qact(j + 2)
                        if sl == 0:
                            if j == 0:
                                for m in (2, 3):
                                    for n in range(LQ // 512):
                                        work.extend(
                                            kproj_chain_mms(kx, n, m))
                            if j < NJ - 1 and (j + 1) not in qxs:
                                qxs[j + 1] = load_qact(j + 1)
                            if j < NJ - 1:
                                for m in range(NMC):
                                    work.extend(
                                        qproj_chain_mms(qxs[j + 1], j + 1, m))
                            if do_c and j >= 1:
                                for m in range(D // 128):
                                    work.extend(outproj_chain_mms(j - 1, m))
                            budget[0] = 0.0
                        budget[0] += 1.5 if j == 0 else 1.0
                        while budget[0] >= 1.0 and work:
                            work.pop(0)()
                            budget[0] -= 1.0
                if do_c:
                    while work:
                        work.pop(0)()
                    for m in range(D // 128):
                        for mm in outproj_chain_mms(NJ - 1, m):
                            mm()
                else:
                    for (c, j), oh in sorted(oht.items()):
                        nc.sync.dma_start(
                            outT[c * 128:(c + 1) * 128,
                                 j * JQ:(j + 1) * JQ],
                            oh[:, 0:JQ // 2].bitcast(F32))

            if iters == 1:
                body()
            else:
                with tc.For_i(0, iters, 1):
                    body()

    nc.compile()
    return nc


def get_program(iters=1, phases="abc", xdve=XDVE):
    key = (iters, phases, xdve)
    if key not in _PROG_CACHE:
        _PROG_CACHE[key] = build_program(iters, phases, xdve)
    return _PROG_CACHE[key]


def shard_inputs(q, k, v, Wq, Wk, Wv, Wo):
    """Build the 8 per-core input maps (host-side layout prep only)."""
    import ml_dtypes
    BF = ml_dtypes.bfloat16
    q, k, v = (np.asarray(x, np.float32) for x in (q, k, v))
    Wq, Wk, Wv, Wo = (np.asarray(x, np.float32) for x in (Wq, Wk, Wv, Wo))
    in_maps = []
    for core in range(N_CORES):
        b, g = core // 2, core % 2
        rows = slice(g * HD_LOC, (g + 1) * HD_LOC)
        in_maps.append({
            "qT": np.ascontiguousarray(q[b].T.astype(BF)),
            "kT": np.ascontiguousarray(k[b].T.astype(BF)),
            "vT": np.ascontiguousarray(v[b].T.astype(BF)),
            "wqT": np.ascontiguousarray(Wq[rows, :].T.astype(BF)),
            "wkT": np.ascontiguousarray(Wk[rows, :].T.astype(BF)),
            "wvT": np.ascontiguousarray(Wv[rows, :].T.astype(BF)),
            "woT": np.ascontiguousarray(Wo[:, rows].T.astype(BF)),
        })
    return in_maps


def gather_outputs(results):
    out = np.empty((B, LQ, D), np.float32)
    for b in range(B):
        acc = results[2 * b]["outT"] + results[2 * b + 1]["outT"]
        out[b] = acc.T
    return out


def kernel(q, k, v, Wq, Wk, Wv, Wo):
    from concourse.bass_utils import run_bass_kernel_spmd

    nc = get_program(1)
    in_maps = shard_inputs(q, k, v, Wq, Wk, Wv, Wo)
    res = run_bass_kernel_spmd(nc, in_maps, core_ids=list(range(N_CORES)))
    return gather_outputs(res.results)


# revision 20
# speedup vs baseline: 1.0139x; 1.0018x over previous
"""Trainium2 Bass kernel for 16-head MHA (B=4, L=2048, D=1024, fp32 in/out).

Sharding: batch x head-group over 8 cores. Core c handles batch c//2 and
heads (c%2)*8 .. (c%2)*8+7 (Megatron column-parallel QKV, row-parallel Wo).
Each core computes a partial output projection; the host sums the two
partials per batch.

All matmul operands are bf16 (fp32 accumulate in PSUM). Measured on this
hardware, fp32r moving operands stream at ~half rate (442ns for N=512 vs
213ns bf16), so bf16 halves Tensor-engine time; rel err stays ~1e-3 vs
the 2e-2 gate.

Per-core device program (SPMD, identical on all cores):
  upfront: VH = v @ Wv_g.T  -> [2048, 8*65] bf16 (65th col of each head =
           1.0 so the AV matmul also yields the softmax denominator);
           KHT = Wk_g @ k.T -> [512, 2048]; Q projection for column 0.
  main loop over 512-wide query columns j, head pairs c, key chunks lk:
      one [128,1024] PSUM score tile holds head A in cols 0:512 and head B
      in cols 512:1024 (two K=64 bf16 matmuls packed in PE row groups 0-1 /
      2-3, running concurrently); one FD=1024 exp converts it to bf16
      attention weights (optionally the tail XDVE columns are computed on
      the Vector engine with a Schraudolph bit-trick exp to offload the
      Activation engine); two K=128 matmuls accumulate O_un (+ denominator
      row) per head. AV trails scores/exp by PIPE steps. The Q projection
      for column j+1 and the output projection for column j-1 are drip-fed
      ONE matmul per step so the PE never bursts/starves the ACT stream
      (column 0 additionally absorbs the K projection for head pairs 2-3
      at 1.5 matmuls per step).
      finalize: softmax denominators bounce through DRAM and come back
      with queries on partitions ([128, 8]) so the Vector-engine
      reciprocal runs at free-dim 8 (~0.2us, not 3.3us); the broadcast
      back across 64 partitions is a second DRAM-bounce DMA;
      O = O_un * (1/denom) runs on GpSimd (SBUF-only engine, otherwise
      idle). Input DMAs are batched into ~128-512KB pieces: big enough
      to amortize the ~585ns per-DMA Sync-engine issue cost, small
      enough to spread transfers across the 16 DMA engines.
"""

import sys

if "/opt/trn_rl_repo" not in sys.path:
    sys.path.insert(0, "/opt/trn_rl_repo")

import numpy as np

B, LQ, LV, D, H = 4, 2048, 2048, 1024, 16
DH = D // H            # 64
N_CORES = 8
H_LOC = H // 2         # 8 heads per core
HD_LOC = H_LOC * DH    # 512 head-dims per core
NKC = D // 128         # 8 contraction chunks for projections
NS = LV // 128         # 16 key chunks
NMC = HD_LOC // 128    # 4 head-dim chunks (head pairs)
JQ = 512               # query block width in attention
NJ = LQ // JQ          # 4 query columns
VW = DH + 1            # 65: per-head V width incl. ones column

XDVE = 128             # e-tile columns computed on DVE (Schraudolph exp)
# bf16-as-int16 Schraudolph: e = exp(0.125*s) = 2^(0.125*log2e*s);
# bf16 bits of 2^y ~= 128*y + 127*128; mean-centering correction 7.42.
SCH_A = 0.125 * 128 * 1.4426950408889634
SCH_B = 16256.0 - 7.42

_PROG_CACHE = {}


def build_program(iters=1, phases="abc", xdve=XDVE):
    import concourse.bass as bass
    import concourse.tile as tile
    from concourse import bacc, mybir

    F32 = mybir.dt.float32
    BF16 = mybir.dt.bfloat16
    I16 = mybir.dt.int16
    EXP = mybir.ActivationFunctionType.Exp
    MULT = mybir.AluOpType.mult
    ADD = mybir.AluOpType.add

    nc = bacc.Bacc("TRN2", target_bir_lowering=False, debug=False,
                   num_devices=N_CORES)

    qT = nc.dram_tensor("qT", [D, LQ], BF16, kind="ExternalInput").ap()
    kT = nc.dram_tensor("kT", [D, LV], BF16, kind="ExternalInput").ap()
    vT = nc.dram_tensor("vT", [D, LV], BF16, kind="ExternalInput").ap()
    wqT = nc.dram_tensor("wqT", [D, HD_LOC], BF16, kind="ExternalInput").ap()
    wkT = nc.dram_tensor("wkT", [D, HD_LOC], BF16, kind="ExternalInput").ap()
    wvT = nc.dram_tensor("wvT", [D, HD_LOC], BF16, kind="ExternalInput").ap()
    woT = nc.dram_tensor("woT", [HD_LOC, D], BF16, kind="ExternalInput").ap()
    outT = nc.dram_tensor("outT", [D, LQ], F32, kind="ExternalOutput").ap()
    # DRAM bounce rows for broadcasting softmax 1/denom across partitions
    dbc = nc.dram_tensor("dbc", [2 * NMC * NJ, JQ], F32).ap()
    dbc2 = nc.dram_tensor("dbc2", [2 * NMC * NJ, JQ], F32).ap()

    with tile.TileContext(nc) as tc:
        with (
            tc.tile_pool(name="persist", bufs=1) as persist,
            tc.tile_pool(name="wq", bufs=1) as wqp,
            tc.tile_pool(name="wk", bufs=1) as wkp,
            tc.tile_pool(name="qact", bufs=3) as qactp,
            tc.tile_pool(name="kact", bufs=1) as kactp,
            tc.tile_pool(name="qhtj", bufs=8) as qhtp,
            tc.tile_pool(name="ohtj", bufs=8) as ohtp,
            tc.tile_pool(name="e", bufs=4) as epool,
            tc.tile_pool(name="smalls", bufs=4) as smalls,
            tc.tile_pool(name="rbcp", bufs=2) as rbcp,
            tc.tile_pool(name="wo", bufs=1) as wop,
            tc.tile_pool(name="outp", bufs=6) as outp,
            tc.tile_pool(name="pss", bufs=2, space="PSUM") as pss,
            tc.tile_pool(name="psav", bufs=2, space="PSUM") as psav,
            tc.tile_pool(name="pspj", bufs=2, space="PSUM") as pspj,
        ):
            def body():
                kht = [persist.tile([128, LV], BF16, tag=f"kht{m}", name=f"kht{m}")
                       for m in range(NMC)]
                vh = [persist.tile([128, H_LOC * VW], BF16, tag=f"vh{s}", name=f"vh{s}")
                      for s in range(NS)]
                ones8 = persist.tile([128, H_LOC], BF16, tag="ones8", name="ones8")
                nc.vector.memset(ones8[:], 1.0)

                W = {}
                qht = {}   # (m, n) -> [128, JQ] tile
                oht = {}   # (c, j) -> [128, JQ] tile

                def load_qact(n):
                    t = qactp.tile([128, NKC, 512], BF16, tag="qact",
                                   name=f"qx{n}")
                    for h in range(2):
                        a = h * (NKC // 2)
                        nc.sync.dma_start(
                            t[:, a:a + NKC // 2, :],
                            bass.AP(tensor=qT.tensor,
                                    offset=a * 128 * LQ + n * 512,
                                    ap=[[LQ, 128], [128 * LQ, NKC // 2],
                                        [1, 512]]))
                    return t

                def qproj_chain_mms(xs, n, m):
                    # one closure per matmul so chains can be drip-fed one
                    # MM per attention step
                    state = {}
                    mms = []
                    for kc in range(NKC):
                        def mm(kc=kc):
                            if kc == 0:
                                state["p"] = pspj.tile([128, 512], F32,
                                                       tag="pspj",
                                                       name=f"qp{n}_{m}")
                            nc.tensor.matmul(
                                state["p"][:],
                                W["q"][:, kc, m * 128:(m + 1) * 128],
                                xs[:, kc, :],
                                start=(kc == 0), stop=(kc == NKC - 1))
                            if kc == NKC - 1:
                                d = qhtp.tile([128, JQ], BF16, tag="qhtj",
                                              name=f"qh{n}_{m}")
                                nc.vector.tensor_copy(out=d[:], in_=state["p"][:])
                                qht[(m, n)] = d
                        mms.append(mm)
                    return mms

                def kproj_chain_mms(kx, n, m):
                    state = {}
                    mms = []
                    for kc in range(NKC):
                        def mm(kc=kc):
                            if kc == 0:
                                state["p"] = pspj.tile([128, 512], F32,
                                                       tag="pspj",
                                                       name=f"kp{n}_{m}")
                            nc.tensor.matmul(
                                state["p"][:],
                                W["k"][:, kc, m * 128:(m + 1) * 128],
                                kx[:, kc, n * 512:(n + 1) * 512],
                                start=(kc == 0), stop=(kc == NKC - 1))
                            if kc == NKC - 1:
                                nc.vector.tensor_copy(
                                    out=kht[m][:, n * 512:(n + 1) * 512],
                                    in_=state["p"][:])
                        mms.append(mm)
                    return mms

                def outproj_chain_mms(j, m):
                    state = {}
                    NWO = HD_LOC // 128
                    mms = []
                    for kc in range(NWO):
                        def mm(kc=kc):
                            if kc == 0:
                                state["p"] = pspj.tile([128, 512], F32,
                                                       tag="pspj",
                                                       name=f"cp{j}_{m}")
                            nc.tensor.matmul(
                                state["p"][:],
                                W["o"][:, kc, m * 128:(m + 1) * 128],
                                oht[(kc, j)][:],
                                start=(kc == 0), stop=(kc == NWO - 1))
                            if kc == NWO - 1:
                                om = outp.tile([128, JQ], F32, tag="om",
                                               name=f"om{j}_{m}")
                                if j == NJ - 1:
                                    nc.scalar.copy(om[:], state["p"][:])
                                else:
                                    nc.vector.tensor_copy(out=om[:],
                                                          in_=state["p"][:])
                                for q2 in range(2):
                                    if j == NJ - 1:
                                        eng = nc.sync if q2 == 0 else nc.scalar
                                    else:
                                        eng = (nc.sync if m % 2 == 0
                                               else nc.gpsimd)
                                    eng.dma_start(
                                        outT[m * 128:(m + 1) * 128,
                                             j * JQ + q2 * 256:
                                             j * JQ + (q2 + 1) * 256],
                                        om[:, q2 * 256:(q2 + 1) * 256])
                                if m == D // 128 - 1:
                                    for c in range(NMC):
                                        oht.pop((c, j))
                        mms.append(mm)
                    return mms

                qxs = {0: None}

                def wload(pool, name, drt, nkc, width, pieces=2):
                    t = pool.tile([128, nkc, width], BF16, tag=name,
                                  name=name)
                    step = nkc // pieces
                    for h in range(pieces):
                        a = h * step
                        nc.sync.dma_start(
                            t[:, a:a + step, :],
                            bass.AP(tensor=drt.tensor,
                                    offset=a * 128 * width,
                                    ap=[[width, 128],
                                        [128 * width, step],
                                        [1, width]]))
                    return t

                # ---------- upfront: V projection, K projection, Q col 0 ----
                with (tc.tile_pool(name="wproj", bufs=1) as wpool,
                      tc.tile_pool(name="vact", bufs=4) as vactp):
                    psA = pspj
                    wvx = wload(wpool, "wv", wvT, NKC, HD_LOC, pieces=NKC)

                    def load_vx(sg):
                        vx = vactp.tile([128, NKC, 512], BF16, tag="vact",
                                        name=f"vx{sg}")
                        for a in range(NKC):
                            nc.sync.dma_start(
                                vx[:, a, :],
                                vT[a * 128:(a + 1) * 128,
                                   sg * 512:(sg + 1) * 512])
                        return vx

                    vxs = [load_vx(sg) for sg in range(NS // 4)]
                    # k: one persistent [128, NKC, LV] tile, 1 DMA per kc
                    kx = kactp.tile([128, NKC, LV], BF16, tag="kx", name="kx")
                    for kc in range(NKC):
                        nc.sync.dma_start(kx[:, kc, :],
                                          kT[kc * 128:(kc + 1) * 128, :])
                    W["k"] = wload(wkp, "wk", wkT, NKC, HD_LOC)
                    for sg in range(NS // 4):
                        vx = vxs[sg]
                        for si in range(4):
                            s = sg * 4 + si
                            p = psA.tile([128, HD_LOC], F32, tag="pspj", name=f"pv{s}")
                            for kc in range(NKC):
                                nc.tensor.matmul(
                                    p[:], vx[:, kc, si * 128:(si + 1) * 128],
                                    wvx[:, kc, :],
                                    start=(kc == 0), stop=(kc == NKC - 1))
                            v3 = vh[s].rearrange("p (h e) -> p h e", e=VW)
                            nc.vector.tensor_copy(
                                out=v3[:, :, DH:VW],
                                in_=ones8.rearrange("p (h o) -> p h o", o=1))
                            nc.vector.tensor_copy(
                                out=v3[:, :, 0:DH],
                                in_=p.rearrange("p (h e) -> p h e", e=DH))

                    # K projection m=0,1 upfront (m=2,3 drip-fed in column 0)
                    for n in range(LQ // 512):
                        for m in range(2):
                            p = psA.tile([128, 512], F32, tag="pspj",
                                         name=f"kp{n}_{m}")
                            for kc in range(NKC):
                                nc.tensor.matmul(
                                    p[:],
                                    W["k"][:, kc, m * 128:(m + 1) * 128],
                                    kx[:, kc, n * 512:(n + 1) * 512],
                                    start=(kc == 0), stop=(kc == NKC - 1))
                            nc.vector.tensor_copy(
                                out=kht[m][:, n * 512:(n + 1) * 512], in_=p[:])

                    # Q projection for column 0
                    W["q"] = wload(wqp, "wq", wqT, NKC, HD_LOC)
                    xs0 = load_qact(0)
                    W["o"] = wload(wop, "wo", woT, HD_LOC // 128, D)
                    for m in range(NMC):
                        p = psA.tile([128, 512], F32, tag="pspj", name=f"qp0_{m}")
                        for kc in range(NKC):
                            nc.tensor.matmul(
                                p[:],
                                W["q"][:, kc, m * 128:(m + 1) * 128],
                                xs0[:, kc, :],
                                start=(kc == 0), stop=(kc == NKC - 1))
                        d = qhtp.tile([128, JQ], BF16, tag="qhtj", name=f"qh0_{m}")
                        nc.vector.tensor_copy(out=d[:], in_=p[:])
                        qht[(m, 0)] = d
                    if NJ > 1:
                        qxs[1] = load_qact(1)

                if "b" not in phases:
                    nc.sync.dma_start(outT[0:128, 0:1024],
                                      kht[0][:, 0:2048].bitcast(F32))
                    return

                # ---------- main loop ----------
                PIPE = 3
                blocks = [(c, j) for j in range(NJ) for c in range(NMC)]
                steps = [(bi, lk) for bi in range(len(blocks))
                         for lk in range(NS)]
                psx = {}
                ets = {}
                budget = [0.0]

                def emit_scores(bi, lk):
                    c, j = blocks[bi]
                    st = pss.tile([128, 2 * JQ], F32, tag="pss",
                                  name=f"st{bi}_{lk}")
                    nc.tensor.matmul(
                        st[:, 0:JQ],
                        kht[c][0:64, lk * 128:(lk + 1) * 128],
                        qht[(c, j)][0:64, :])
                    nc.tensor.matmul(
                        st[:, JQ:2 * JQ],
                        kht[c][64:128, lk * 128:(lk + 1) * 128],
                        qht[(c, j)][64:128, :])
                    e = epool.tile([128, 2 * JQ], BF16, tag="e",
                                   name=f"e{bi}_{lk}")
                    if xdve:
                        # alternate the approx window between the two tile
                        # edges so every query mixes exact and approx weights
                        if lk % 2 == 0:
                            dlo, dhi = 0, xdve
                            alo, ahi = xdve, 2 * JQ
                        else:
                            dlo, dhi = 2 * JQ - xdve, 2 * JQ
                            alo, ahi = 0, 2 * JQ - xdve
                        nc.scalar.activation(e[:, alo:ahi], st[:, alo:ahi],
                                             EXP, scale=0.125)
                        nc.vector.tensor_scalar(
                            out=e[:, dlo:dhi].bitcast(I16),
                            in0=st[:, dlo:dhi],
                            scalar1=SCH_A, scalar2=SCH_B,
                            op0=MULT, op1=ADD)
                    else:
                        nc.scalar.activation(e[:], st[:], EXP, scale=0.125)
                    ets[(bi, lk)] = e

                def emit_av(bi, lk):
                    c, j = blocks[bi]
                    hA, hB = 2 * c, 2 * c + 1
                    if lk == 0:
                        psx[bi] = (
                            psav.tile([VW, JQ], F32, tag="psav", name=f"pa{bi}"),
                            psav.tile([VW, JQ], F32, tag="psav", name=f"pb{bi}"),
                        )
                    e = ets.pop((bi, lk))
                    pA, pB = psx[bi]
                    nc.tensor.matmul(
                        pA[:], vh[lk][:, hA * VW:(hA + 1) * VW], e[:, 0:JQ],
                        start=(lk == 0), stop=(lk == NS - 1))
                    nc.tensor.matmul(
                        pB[:], vh[lk][:, hB * VW:(hB + 1) * VW], e[:, JQ:2 * JQ],
                        start=(lk == 0), stop=(lk == NS - 1))
                    if lk == NS - 1:
                        emit_finalize(bi)

                def emit_finalize(bi):
                    c, j = blocks[bi]
                    oh = ohtp.tile([128, JQ], BF16, tag="ohtj", name=f"oh{bi}")
                    oht[(c, j)] = oh
                    ouns = []
                    for hi, px in enumerate(psx.pop(bi)):
                        oun = rbcp.tile([VW, JQ], F32, tag="oun",
                                        name=f"ou{bi}_{hi}")
                        nc.vector.tensor_copy(out=oun[:], in_=px[:])
                        di = bi * 2 + hi
                        nc.sync.dma_start(dbc[di:di + 1, :], oun[DH:VW, :])
                        ouns.append(oun)
                    # 1/denom with queries on partitions: flat [128, 8] view
                    # of both rows so the reciprocal has free-dim 8, not 512
                    dt_ = smalls.tile([128, 8], F32, tag="dt", name=f"dt{bi}")
                    nc.sync.dma_start(
                        dt_[:], bass.AP(tensor=dbc.tensor, offset=2 * bi * JQ,
                                        ap=[[8, 128], [1, 8]]))
                    rt = smalls.tile([128, 8], F32, tag="rt", name=f"rt{bi}")
                    nc.vector.reciprocal(rt[:], dt_[:])
                    nc.sync.dma_start(
                        bass.AP(tensor=dbc2.tensor, offset=2 * bi * JQ,
                                ap=[[8, 128], [1, 8]]), rt[:])
                    mul_eng = (nc.vector if bi == NMC * NJ - 1
                               else nc.gpsimd)
                    for hi in range(2):
                        di = bi * 2 + hi
                        rbc = rbcp.tile([64, JQ], F32, tag="rbc",
                                        name=f"rb{bi}_{hi}")
                        bc_src = bass.AP(tensor=dbc2.tensor, offset=di * JQ,
                                         ap=[[0, 64], [1, JQ]])
                        nc.sync.dma_start(rbc[:], bc_src)
                        mul_eng.tensor_mul(
                            oh[hi * 64:(hi + 1) * 64, :],
                            ouns[hi][0:DH, :], rbc[:])

                do_c = "c" in phases
                work = []   # drip-fed projection matmuls, one per step
                for t in range(len(steps) + PIPE):
                    if t < len(steps):
                        emit_scores(*steps[t])
                    if t >= PIPE:
                        bi, lk = steps[t - PIPE]
                        c, j = blocks[bi]
                        emit_av(bi, lk)
                        sl = (bi % NMC) * NS + lk   # step within column
                        if sl == NS and j + 2 < NJ:
                            # prefetch column j+2's activations early
                            qxs[j + 2] = load_qact(j + 2)
                        if sl == 0:
                            if j == 0:
                                for m in (2, 3):
                                    for n in range(LQ // 512):
                                        work.extend(
                                            kproj_chain_mms(kx, n, m))
                            if j < NJ - 1 and (j + 1) not in qxs:
                                qxs[j + 1] = load_qact(j + 1)
                            if j < NJ - 1:
                                for m in range(NMC):
                                    work.extend(
                                        qproj_chain_mms(qxs[j + 1], j + 1, m))
                            if do_c and j >= 1:
                                for m in range(D // 128):
                                    work.extend(outproj_chain_mms(j - 1, m))
                            budget[0] = 0.0
                        budget[0] += 1.5 if j == 0 else 1.0
                        while budget[0] >= 1.0 and work:
                            work.pop(0)()
                            budget[0] -= 1.0
                if do_c:
                    while work:
                        work.pop(0)()
                    for m in range(D // 128):
                        for mm in outproj_chain_mms(NJ - 1, m):
                            mm()
                else:
                    for (c, j), oh in sorted(oht.items()):
                        nc.sync.dma_start(
                            outT[c * 128:(c + 1) * 128,
                                 j * JQ:(j + 1) * JQ],
                            oh[:, 0:JQ // 2].bitcast(F32))

            if iters == 1:
                body()
            else:
                with tc.For_i(0, iters, 1):
                    body()

    nc.compile()
    return nc


def get_program(iters=1, phases="abc", xdve=XDVE):
    key = (iters, phases, xdve)
    if key not in _PROG_CACHE:
        _PROG_CACHE[key] = build_program(iters, phases, xdve)
    return _PROG_CACHE[key]


def shard_inputs(q, k, v, Wq, Wk, Wv, Wo):
    """Build the 8 per-core input maps (host-side layout prep only)."""
    import ml_dtypes
    BF = ml_dtypes.bfloat16
    q, k, v = (np.asarray(x, np.float32) for x in (q, k, v))
    Wq, Wk, Wv, Wo = (np.asarray(x, np.float32) for x in (Wq, Wk, Wv, Wo))
    in_maps = []
    for core in range(N_CORES):
        b, g = core // 2, core % 2
        rows = slice(g * HD_LOC, (g + 1) * HD_LOC)
        in_maps.append({
            "qT": np.ascontiguousarray(q[b].T.astype(BF)),
            "kT": np.ascontiguousarray(k[b].T.astype(BF)),
            "vT": np.ascontiguousarray(v[b].T.astype(BF)),
            "wqT": np.ascontiguousarray(Wq[rows, :].T.astype(BF)),
            "wkT": np.ascontiguousarray(Wk[rows, :].T.astype(BF)),
            "wvT": np.ascontiguousarray(Wv[rows, :].T.astype(BF)),
            "woT": np.ascontiguousarray(Wo[:, rows].T.astype(BF)),
        })
    return in_maps


def gather_outputs(results):
    out = np.empty((B, LQ, D), np.float32)
    for b in range(B):
        acc = results[2 * b]["outT"] + results[2 * b + 1]["outT"]
        out[b] = acc.T
    return out


def kernel(q, k, v, Wq, Wk, Wv, Wo):
    from concourse.bass_utils import run_bass_kernel_spmd

    nc = get_program(1)
    in_maps = shard_inputs(q, k, v, Wq, Wk, Wv, Wo)
    res = run_bass_kernel_spmd(nc, in_maps, core_ids=list(range(N_CORES)))
    return gather_outputs(res.results)
